# revision 27
# baseline (speedup 1.0000x reference)
"""Trainium2 Bass kernel for nn_DenseExpert (soft-gated mixture of dense experts).

Math:  out[b,u] = sum_e gate[b,e] * (x[b,:] @ alpha[e]) [u] + (gate @ beta)[b,u]

Strategy (pure data parallel over batch, 8 cores). Per 512-row chunk per core:
  1. All inputs arrive pre-cast to fp16 from the host: x, gate.T, alpha
     (rearranged [I, E, U]), beta, and the block-diagonal gate stack
     dstack[r, e, c] = gate[r,e]*[c == r%64] (a pure host-side scatter).
     x streams in per-chunk (2 chunks ahead) and dstack per-chunk as one
     merged transfer, both on the sync HWDGE queue; a short run of dummy
     matmuls warms the PE (HAM un-throttle) while the first DMAs land.
  2. y_e.T tiles via PE matmuls: for each 64-row block l of a 128-row tile,
     yT[i, (e, c)] = x[64l:64l+64, :].T @ dstack[64l:64l+64]   (N=512).
     The two 64-row blocks auto-place in distinct PE row groups
     (tile_position h0/h64) and stream concurrently.
  3. PSUM->SBUF copies gather yT into fp16 [i, e, b] layout (DVE/ACT
     alternating per tile).
  4. PE matmuls accumulate out.T[u,b]: the K=8 bias matmul beta.T @ gate.T
     opens the PSUM group (its weight load hides under stage-1 streams),
     then 8 accumulating matmuls alpha_e.T @ y_e.T (N=512, contiguous
     moving reads).
  5. out.T (fp32) copied to SBUF (DVE/ACT split) and DMA'd to DRAM in
     [U, B] layout; the host transposes when assembling the full result.
"""
import dataclasses
from contextlib import ExitStack

import numpy as np

import concourse.bacc as bacc
import concourse.tile as tile
import concourse.mybir as mybir
from concourse.bass_utils import run_bass_kernel_spmd

F32 = mybir.dt.float32
F16 = mybir.dt.float16

B, E, I, U = 65536, 8, 128, 128
NCORES = 8
BLOC = B // NCORES        # 8192 batch rows per core
CHUNK = 512               # batch rows per pipeline chunk
NCHUNK = BLOC // CHUNK    # 16
TPC = CHUNK // 128        # 128-row tiles per chunk
KB = 64                   # contraction block for the diag trick


def _build():
    nc = bacc.Bacc("TRN2", target_bir_lowering=False, debug=False)

    x = nc.dram_tensor("x", [BLOC, I], F16, kind="ExternalInput").ap()
    gateT = nc.dram_tensor("gateT", [E, BLOC], F16, kind="ExternalInput").ap()
    alpha = nc.dram_tensor("alpha", [I, E, U], F16, kind="ExternalInput").ap()
    beta = nc.dram_tensor("beta", [E, U], F16, kind="ExternalInput").ap()
    dstack = nc.dram_tensor("dstack", [BLOC, E, KB], F16, kind="ExternalInput").ap()
    # output stays feature-major on HW; host transposes when assembling
    outT = nc.dram_tensor("outT", [U, BLOC], F32, kind="ExternalOutput").ap()

    with tile.TileContext(nc) as tc, ExitStack() as ctx:
        const = ctx.enter_context(tc.tile_pool(name="const", bufs=1))
        dgp = ctx.enter_context(tc.tile_pool(name="dgp", bufs=4))
        ytp = ctx.enter_context(tc.tile_pool(name="ytp", bufs=3))
        op = ctx.enter_context(tc.tile_pool(name="op", bufs=3))
        ps_yt = ctx.enter_context(tc.tile_pool(name="ps_yt", bufs=3, space="PSUM"))
        ps_ot = ctx.enter_context(tc.tile_pool(name="ps_ot", bufs=2, space="PSUM"))

        # --- resident constants/input tiles (all fp16 from host) ---
        # x chunks first on the sync queue (chunk 0's matmuls gate on them);
        # small constants go on the scalar HWDGE queue.
        xp = ctx.enter_context(tc.tile_pool(name="xp", bufs=4))

        def load_x(c):
            x_h = xp.tile([128, TPC, I], F16, tag="xh")
            nc.sync.dma_start(
                x_h[:],
                x[c * CHUNK : (c + 1) * CHUNK, :].rearrange(
                    "(t p) i -> p t i", p=128
                ),
            )
            return x_h

        def load_diag(c):
            diag_c = dgp.tile([128, TPC, E, KB], F16, tag="diag")
            nc.sync.dma_start(
                diag_c[:],
                dstack[c * CHUNK : (c + 1) * CHUNK, :, :].rearrange(
                    "(t p) e k -> p t e k", p=128
                ),
            )
            return diag_c

        x0 = load_x(0)
        d0 = load_diag(0)
        x1 = load_x(1)
        d1 = load_diag(1)
        alpha_h = const.tile([128, E, U], F16, tag="alphah")
        nc.sync.dma_start(alpha_h[:], alpha)
        beta_h = const.tile([8, U], F16, tag="betah")
        nc.sync.dma_start(beta_h[:], beta)
        gT_h = const.tile([8, BLOC], F16, tag="gTh")
        nc.sync.dma_start(gT_h[:], gateT)
        xq = {0: x0, 1: x1}
        dq = {0: d0, 1: d1}

        # PE warmup: dummy matmuls on a memset scratch tile keep the PE busy
        # (and HAM-warm) while the first real chunk's DMAs land.
        warm = const.tile([64, 512], F16, tag="warm")
        nc.vector.memset(warm[:], 0.0)
        warm_ps = ps_ot.tile([128, CHUNK], F32, tag="oTps")
        for w in range(8):
            nc.tensor.matmul(
                warm_ps[:],
                warm[:, :128],
                warm[:],
                start=(w == 0),
                stop=(w == 7),
            )

        def emit_front(c):
            # per 128-row tile: diag DMA (gpsimd queue) + yT matmuls + copy
            x_h = xq.pop(c)
            diag_c = dq.pop(c)
            if c + 2 < NCHUNK:
                xq[c + 2] = load_x(c + 2)
                dq[c + 2] = load_diag(c + 2)
            yT_all = ytp.tile([128, E, TPC, 128], F16, tag="yT")
            for t in range(TPC):
                yT_ps = ps_yt.tile([128, 2, E, KB], F32, tag="yTps")
                for l in range(2):
                    nc.tensor.matmul(
                        yT_ps[:, l, :, :],
                        x_h[l * KB : (l + 1) * KB, t, :],
                        diag_c[l * KB : (l + 1) * KB, t, :, :],
                        start=True,
                        stop=True,
                    )
                dst = dataclasses.replace(
                    yT_all[:],
                    ap=[[E * TPC * 128, 128], [KB, 2], [TPC * 128, E], [1, KB]],
                    offset=t * 128,
                )
                if t % 2 == 0:
                    nc.vector.tensor_copy(dst, yT_ps[:])
                else:
                    nc.scalar.copy(dst, yT_ps[:])
            return yT_all

        def emit_back(c, yT_all):
            row0 = c * CHUNK
            oT_ps = ps_ot.tile([128, CHUNK], F32, tag="oTps")
            nc.tensor.matmul(
                oT_ps[:],
                beta_h[:],
                gT_h[:, row0 : row0 + CHUNK],
                start=True,
                stop=False,
            )
            for e in range(E):
                nc.tensor.matmul(
                    oT_ps[:],
                    alpha_h[:, e, :],
                    yT_all[:, e, :, :],
                    start=False,
                    stop=(e == E - 1),
                )

            oT_sb = op.tile([128, CHUNK], F32, tag="oT")
            nc.vector.tensor_copy(oT_sb[:, : CHUNK // 2], oT_ps[:, : CHUNK // 2])
            nc.scalar.copy(oT_sb[:, CHUNK // 2 :], oT_ps[:, CHUNK // 2 :])
            nc.sync.dma_start(outT[:, row0 : row0 + CHUNK], oT_sb[:])

        pending = None
        for c in range(NCHUNK):
            front = emit_front(c)
            if pending is not None:
                emit_back(c - 1, pending)
            pending = front
        emit_back(NCHUNK - 1, pending)

    nc.compile()
    return nc


_NC_CACHE = None


def make_in_maps(x, gate_perc, alpha, beta):
    x = np.asarray(x, dtype=np.float32).astype(np.float16)
    gate_perc = np.asarray(gate_perc, dtype=np.float32).astype(np.float16)
    alpha16 = np.ascontiguousarray(
        np.asarray(alpha, dtype=np.float32).astype(np.float16).transpose(1, 0, 2)
    )
    beta16 = np.asarray(beta, dtype=np.float32).astype(np.float16)
    # dstack[r, c, e] = gate[r, e] * [c == r % KB]   (pure scatter)
    dstack = np.zeros((B, E, KB), np.float16)
    rows = np.arange(B)
    dstack[rows, :, rows % KB] = gate_perc
    in_maps = []
    for c in range(NCORES):
        sl = slice(c * BLOC, (c + 1) * BLOC)
        in_maps.append(
            {
                "x": np.ascontiguousarray(x[sl]),
                "gateT": np.ascontiguousarray(gate_perc[sl].T),
                "alpha": alpha16,
                "beta": beta16,
                "dstack": dstack[sl],
            }
        )
    return in_maps


def kernel(x, gate_perc, alpha, beta):
    global _NC_CACHE
    if _NC_CACHE is None:
        _NC_CACHE = _build()
    nc = _NC_CACHE

    in_maps = make_in_maps(x, gate_perc, alpha, beta)
    res = run_bass_kernel_spmd(nc, in_maps, list(range(NCORES))).results
    # per-core outputs are [U, BLOC]; assemble and transpose on host
    full_T = np.concatenate([res[c]["outT"] for c in range(NCORES)], axis=1)
    return np.ascontiguousarray(full_T.T)


if __name__ == "__main__":
    rng = np.random.default_rng(0)
    x = rng.standard_normal((B, I)).astype(np.float32)
    g = rng.random((B, E)).astype(np.float32)
    g /= g.sum(-1, keepdims=True)
    al = (rng.standard_normal((E, I, U)) * 0.05).astype(np.float32)
    be = (rng.standard_normal((E, U)) * 0.05).astype(np.float32)
    got = kernel(x, g, al, be)
    ref = np.einsum("bi,eio->beo", x, al, optimize=True)
    ref = np.einsum("beo,be->bo", ref, g) + g @ be
    err = np.abs(got - ref)
    print("max abs err", err.max(), "rel", err.max() / np.abs(ref).max())
